# revision 18
# baseline (speedup 1.0000x reference)
"""Trainium2 Bass kernel for nn_CANLayer (CAN GNN layer).

Strategy (8-core SPMD, no collectives):
  - Targets are sharded: core k owns target nodes [k*6250, (k+1)*6250).
    Edges are routed to the core owning their target, so every segment
    (softmax group) is fully local to one core.
  - Per conv branch, out = relu-later( (1/s_t) * (sum_e w_e * x[src_e]) @ W )
    with w_e = exp(elu(st[tgt_e] + ss[src_e]) * val_e),
    st = x @ (W a1), ss = x @ (W a2)  (projection commutes with the
    softmax-weighted segment sum, so we gather raw x rows, not x@W).
  - Edges sorted by target, tiled into 128-target windows.  For each
    128-edge chunk a weighted one-hot matrix maskw[e, t] = (iota==tgt_off)*w
    is built in one fused vector op; the tensor engine then computes
    U[t, :] += maskw^T @ Xg and s[t] += maskw^T @ 1 (PSUM accumulation).
  - x rows are fetched with dma_gather (512B bf16 rows, int16 indices split
    into two base-offset gathers), st/ss are fetched per edge with 4-byte
    indirect DMA gathers from a device-computed [4, Npad] score table.
  - Final: out = relu(A_l @ W_l + A_u @ W_u + x_own @ (W_lin*EPS)), with
    A (normalized segment sums) DMA-transposed for the projection matmuls.
"""

import math
import os
import sys
from contextlib import ExitStack

import numpy as np
import ml_dtypes

for _p in ("/opt/trn_rl_repo", "/root/.axon_site/_ro/trn_rl_repo"):
    if os.path.isdir(_p) and _p not in sys.path:
        sys.path.insert(0, _p)

import concourse.bass as bass
import concourse.bacc as bacc
import concourse.tile as tile
from concourse import mybir
from concourse import library_config
from concourse.bass import IndirectOffsetOnAxis
from concourse.bass_utils import run_bass_kernel_spmd

BF16 = mybir.dt.bfloat16
F32 = mybir.dt.float32
I16 = mybir.dt.int16
I32 = mybir.dt.int32
ALU = mybir.AluOpType
ACTF = mybir.ActivationFunctionType

EPS = 1.0 + 1e-06
PAD_OFF = 200.0  # tgt_off value for pad edge slots (outside [0,128) window)


# --------------------------------------------------------------------------
# host-side preprocessing
# --------------------------------------------------------------------------

def _ceil(a, b):
    return -(-a // b)


def _prep_conv(indices, values, n, n_cores, own, ntiles, split):
    """Sort/tile/pad one conv's edges.  Returns per-core arrays + baked meta.

    Per tile (128 consecutive owned targets): edges are grouped as
    [A-edges (src < split) | pad | B-edges | pad], each padded up to a
    multiple of 128 with (src=0, val=0, tgt_off=PAD_OFF).  Chunk counts are
    maxed over cores so one SPMD program fits all cores.
    """
    tgt = np.asarray(indices[0], dtype=np.int64).astype(np.int32)
    src = np.asarray(indices[1], dtype=np.int64).astype(np.int32)
    val = np.asarray(values, dtype=np.float32)

    core_of = tgt // own
    ownp = ntiles * 128

    # per (core, tile): lists of edge indices, split by A/B
    per_core = []
    for k in range(n_cores):
        sel = np.nonzero(core_of == k)[0]
        tl = tgt[sel] - k * own          # local target id
        order = np.argsort(tl, kind="stable")
        sel = sel[order]
        tl = tl[order]
        tile_id = tl >> 7
        bounds = np.searchsorted(tile_id, np.arange(ntiles + 1))
        tiles = []
        for t in range(ntiles):
            e = sel[bounds[t]:bounds[t + 1]]
            isa = src[e] < split
            tiles.append((e[isa], e[~isa]))
        per_core.append(tiles)

    ncha = [max(_ceil(max(len(per_core[k][t][0]) for k in range(n_cores)), 128), 1)
            for t in range(ntiles)]
    nchb = [_ceil(max(len(per_core[k][t][1]) for k in range(n_cores)), 128)
            for t in range(ntiles)]
    ch = [a + b for a, b in zip(ncha, nchb)]
    cht = sum(ch)
    fa = sum(ncha) * 8   # int16 idx cols (128 idx -> 8 cols of 16)
    fb = sum(nchb) * 8

    def wrap_idx(vals16, out, col0):
        # linear idx i -> (partition i%16 [+16*g replicas], col i//16)
        m = len(vals16) // 16
        blk = vals16.reshape(m, 16).T  # [16, m]
        for g in range(8):
            out[g * 16:(g + 1) * 16, col0:col0 + m] = blk

    cores = []
    for k in range(n_cores):
        idxa = np.zeros((128, fa), np.int16)
        idxb = np.zeros((128, max(fb, 1)), np.int16)
        toff = np.full((128, cht), PAD_OFF, np.float32)
        vals = np.zeros((128, cht), np.float32)
        toffi = np.zeros((128, cht * 8), np.int16)
        ca = cb = cc = 0
        for t in range(ntiles):
            ea, eb = per_core[k][t]
            for which, e, nch in (("a", ea, ncha[t]), ("b", eb, nchb[t])):
                nslot = nch * 128
                s = np.zeros(nslot, np.int32)
                s[:len(e)] = src[e] if which == "a" else src[e] - split
                to = np.full(nslot, PAD_OFF, np.float32)
                to[:len(e)] = (tgt[e] - k * own - t * 128).astype(np.float32)
                vv = np.zeros(nslot, np.float32)
                vv[:len(e)] = val[e]
                # chunk-major [p, c] layout: slot i -> (i % 128, i // 128)
                cols = slice(cc, cc + nch)
                toff[:, cols] = to.reshape(nch, 128).T
                vals[:, cols] = vv.reshape(nch, 128).T
                ti = np.zeros(nslot, np.int32)
                ti[:len(e)] = (tgt[e] - k * own - t * 128).astype(np.int32)
                wrap_idx(ti.astype(np.int16), toffi, cc * 8)
                cc += nch
                if which == "a":
                    wrap_idx(s.astype(np.int16), idxa, ca * 8)
                    ca += nch
                else:
                    if nch:
                        wrap_idx(s.astype(np.int16), idxb, cb * 8)
                    cb += nch
        cores.append(dict(idxa=idxa, idxb=idxb, toff=toff, val=vals,
                          toffi=toffi))

    meta = dict(ncha=ncha, nchb=nchb, ch=ch, cht=cht, fa=fa, fb=max(fb, 1),
                ownp=ownp)
    return meta, cores


def _build_program(n, npad, d, n_cores, own, ntiles, split, meta_l, meta_u):
    """Build the single SPMD Bass/Tile program."""
    nc = bacc.Bacc(trn_type="TRN2", target_bir_lowering=False, debug=False,
                    num_devices=n_cores, num_swdge_queues=4)
    ownp = ntiles * 128

    def din(name, shape, dt):
        return nc.dram_tensor(name, shape, dt, kind="ExternalInput")

    x_tab = din("x_tab", [npad, d], BF16)
    xt = din("xt", [d, npad], BF16)
    xot = din("xot", [d, ownp], BF16)
    wa4 = din("wa4", [d, 4], BF16)
    w_l = din("w_l", [d, d], BF16)
    w_u = din("w_u", [d, d], BF16)
    w_lin = din("w_lin", [d, d], BF16)
    iota_in = din("iota_in", [128, 128], F32)
    convs = {}
    for cv, meta in (("l", meta_l), ("u", meta_u)):
        convs[cv] = dict(
            meta=meta,
            idxa=din(f"idxa_{cv}", [128, meta["fa"]], I16),
            idxb=din(f"idxb_{cv}", [128, meta["fb"]], I16),
            toff=din(f"toff_{cv}", [128, meta["cht"]], F32),
            val=din(f"val_{cv}", [128, meta["cht"]], F32),
            toffi=din(f"toffi_{cv}", [128, meta["cht"] * 8], I16),
            a_dram=nc.dram_tensor(f"a_{cv}", [ownp, d], BF16),
        )
    sc_tab = nc.dram_tensor("sc_tab", [npad, 64], F32)
    st_tab = nc.dram_tensor("st_tab", [ownp, 64], F32)
    out = nc.dram_tensor("out", [ownp, d], F32, kind="ExternalOutput")

    mvj = 512  # matvec rhs tile cols
    nmv = npad // mvj
    gmax = 8  # max chunks (x128 idx) per dma_gather call

    def split_gather(out_tile, co, nch, table, idx_sb, io, elem):
        g0 = 0
        while g0 < nch:
            g = min(gmax, nch - g0)
            nc.gpsimd.dma_gather(
                out_tile[:, co + g0:co + g0 + g, :], table,
                idx_sb[:, (io + g0) * 8:(io + g0 + g) * 8],
                g * 128, g * 128, elem, elem_step=elem, queue_num=0)
            g0 += g

    with tile.TileContext(nc) as tc:
        with ExitStack() as ctx:
            pool = ctx.enter_context(tc.tile_pool(name="sb", bufs=4))
            cpool = ctx.enter_context(tc.tile_pool(name="const", bufs=1))
            psum = ctx.enter_context(tc.tile_pool(name="ps", bufs=2, space="PSUM"))

            iota_t = cpool.tile([128, 128], F32)
            nc.sync.dma_start(iota_t[:], iota_in[:, :])
            ones_t = cpool.tile([128, 1], BF16)
            nc.vector.memset(ones_t[:], 1.0)
            wa_sb = cpool.tile([128, 2, 4], BF16)
            nc.sync.dma_start(wa_sb[:, 0, :], wa4[0:128, :])
            nc.sync.dma_start(wa_sb[:, 1, :], wa4[128:256, :])

            # ---- phase 0: zero the score tables --------------------------
            zt = cpool.tile([128, 1024], F32)
            nc.vector.memset(zt[:], 0.0)
            zchunk = 128 * 1024  # elements per zeroing DMA
            flat_sc = sc_tab[:, :].rearrange("a b -> (a b)")
            for z0 in range(0, npad * 64, zchunk):
                zn = min(zchunk, npad * 64 - z0)
                nc.sync.dma_start(flat_sc[z0:z0 + zn].rearrange(
                    "(p f) -> p f", p=128), zt[:, 0:zn // 128])
            flat_st = st_tab[:, :].rearrange("a b -> (a b)")
            for z0 in range(0, ownp * 64, zchunk):
                zn = min(zchunk, ownp * 64 - z0)
                nc.sync.dma_start(flat_st[z0:z0 + zn].rearrange(
                    "(p f) -> p f", p=128), zt[:, 0:zn // 128])

            # ---- phase 1: score tables via matvec + 32x128 transposes ----
            # sc_tab rows: [ss_l, st_l, ss_u, st_u, 0...]; st_tab: [st_l, st_u]
            for j in range(nmv):
                xt_t = pool.tile([128, 2, mvj], BF16, tag="xtile")
                nc.sync.dma_start(xt_t[:, 0, :], xt[0:128, j * mvj:(j + 1) * mvj])
                nc.sync.dma_start(xt_t[:, 1, :], xt[128:256, j * mvj:(j + 1) * mvj])
                mv_ps = psum.tile([4, mvj], F32, tag="mvps", bufs=2)
                nc.tensor.matmul(out=mv_ps[:], lhsT=wa_sb[:, 0, :],
                                 rhs=xt_t[:, 0, :], start=True, stop=False)
                nc.tensor.matmul(out=mv_ps[:], lhsT=wa_sb[:, 1, :],
                                 rhs=xt_t[:, 1, :], start=False, stop=True)
                mv_sb = pool.tile([32, mvj], F32, tag="mvsb")
                nc.vector.memset(mv_sb[:], 0.0)
                nc.vector.tensor_copy(out=mv_sb[0:4, :], in_=mv_ps[:])
                for c in range(mvj // 128):
                    tr = pool.tile([32, 128], F32, tag="mvtr")
                    nc.vector.transpose(tr[:], mv_sb[:, c * 128:(c + 1) * 128])
                    j0c = j * mvj + c * 128
                    nc.sync.dma_start(
                        sc_tab[j0c:j0c + 128, 0:4].rearrange(
                            "(b r) f -> r b f", b=4),
                        tr[:].rearrange("r (b f) -> r b f", b=4)[:, :, 0:4])

            # own-target st table from xot: rows [st_l, st_u]
            nmvo = ownp // mvj if ownp % mvj == 0 else ownp // mvj + 1
            for j in range(nmvo):
                w0 = j * mvj
                w1 = min(w0 + mvj, ownp)
                wn = w1 - w0
                xo_t = pool.tile([128, 2, wn], BF16, tag="xotile")
                nc.sync.dma_start(xo_t[:, 0, :], xot[0:128, w0:w1])
                nc.sync.dma_start(xo_t[:, 1, :], xot[128:256, w0:w1])
                mo_ps = psum.tile([2, wn], F32, tag="mops", bufs=1)
                nc.tensor.matmul(out=mo_ps[:], lhsT=wa_sb[:, 0, 2:4],
                                 rhs=xo_t[:, 0, :], start=True, stop=False)
                nc.tensor.matmul(out=mo_ps[:], lhsT=wa_sb[:, 1, 2:4],
                                 rhs=xo_t[:, 1, :], start=False, stop=True)
                mo_sb = pool.tile([32, wn], F32, tag="mosb")
                nc.vector.memset(mo_sb[:], 0.0)
                nc.vector.tensor_copy(out=mo_sb[0:2, :], in_=mo_ps[:])
                for c in range(wn // 128):
                    tro = pool.tile([32, 128], F32, tag="motr")
                    nc.vector.transpose(tro[:], mo_sb[:, c * 128:(c + 1) * 128])
                    w0c = w0 + c * 128
                    nc.sync.dma_start(
                        st_tab[w0c:w0c + 128, 0:2].rearrange(
                            "(b r) f -> r b f", b=4),
                        tro[:].rearrange("r (b f) -> r b f", b=4)[:, :, 0:2])

            tc.strict_bb_all_engine_barrier()

            # ---- phase 2: per-conv edge processing -----------------------
            for cvi, cv in enumerate(("l", "u")):
                cd = convs[cv]
                meta = cd["meta"]
                ncha, nchb, chs = meta["ncha"], meta["nchb"], meta["ch"]
                offa = np.cumsum([0] + ncha)
                offb = np.cumsum([0] + nchb)
                offc = np.cumsum([0] + chs)
                ss_f = cvi          # field in sc_tab (ss_l=0, ss_u=1)
                st_f = cvi          # field in st_tab (st_l=0, st_u=1)
                for t in range(ntiles):
                    na, nb, ch = ncha[t], nchb[t], chs[t]
                    c0 = offc[t]
                    xg = pool.tile([128, ch, d], BF16, tag="xg")
                    ia = pool.tile([128, na * 8], I16, tag="ia")
                    nc.sync.dma_start(ia[:], cd["idxa"][:, offa[t] * 8:(offa[t] + na) * 8])
                    split_gather(xg, 0, na, x_tab[:, :], ia, 0, d)
                    if nb:
                        ib = pool.tile([128, nb * 8], I16, tag="ib")
                        nc.sync.dma_start(ib[:], cd["idxb"][:, offb[t] * 8:(offb[t] + nb) * 8])
                        split_gather(xg, na, nb, x_tab[split:, :], ib, 0, d)

                    ssg = pool.tile([128, ch, 64], F32, tag="ssg")
                    split_gather(ssg, 0, na, sc_tab[:, :], ia, 0, 64)
                    if nb:
                        split_gather(ssg, na, nb, sc_tab[split:, :], ib, 0, 64)
                    ti = pool.tile([128, ch * 8], I16, tag="ti")
                    nc.sync.dma_start(ti[:], cd["toffi"][:, c0 * 8:(c0 + ch) * 8])
                    stg = pool.tile([128, ch, 64], F32, tag="stg")
                    split_gather(stg, 0, ch, st_tab[t * 128:, :], ti, 0, 64)
                    ss = ssg[:, :, ss_f:ss_f + 1]
                    st = stg[:, :, st_f:st_f + 1]

                    tof = pool.tile([128, ch], F32, tag="tof")
                    nc.sync.dma_start(tof[:], cd["toff"][:, c0:c0 + ch])
                    vv = pool.tile([128, ch], F32, tag="vv")
                    nc.sync.dma_start(vv[:], cd["val"][:, c0:c0 + ch])

                    # w = exp(elu(st+ss) * val)
                    z = pool.tile([128, ch], F32, tag="z")
                    nc.vector.tensor_add(out=z[:], in0=st, in1=ss)
                    zm = pool.tile([128, ch], F32, tag="zm")
                    nc.vector.tensor_scalar(out=zm[:], in0=z[:], scalar1=0.0,
                                            scalar2=None, op0=ALU.min)
                    e1 = pool.tile([128, ch], F32, tag="e1")
                    nc.scalar.activation(e1[:], zm[:], ACTF.Exp)
                    zp = pool.tile([128, ch], F32, tag="zp")
                    nc.vector.tensor_scalar(out=zp[:], in0=z[:], scalar1=0.0,
                                            scalar2=None, op0=ALU.max)
                    t1 = pool.tile([128, ch], F32, tag="t1")
                    nc.vector.tensor_add(out=t1[:], in0=zp[:], in1=e1[:])
                    t2 = pool.tile([128, ch], F32, tag="t2")
                    nc.vector.tensor_mul(out=t2[:], in0=t1[:], in1=vv[:])
                    v = pool.tile([128, ch], F32, tag="v")
                    nc.vector.tensor_sub(out=v[:], in0=t2[:], in1=vv[:])
                    w = pool.tile([128, ch], F32, tag="w")
                    nc.scalar.activation(w[:], v[:], ACTF.Exp)

                    u_ps = psum.tile([128, d], F32, tag="ups", bufs=2)
                    s_ps = psum.tile([128, 1], F32, tag="sps", bufs=1)
                    for c in range(ch):
                        mw = pool.tile([128, 128], BF16, tag="mw", bufs=8)
                        nc.vector.tensor_scalar(
                            out=mw[:], in0=iota_t[:], scalar1=tof[:, c:c + 1],
                            scalar2=w[:, c:c + 1], op0=ALU.is_equal, op1=ALU.mult)
                        nc.tensor.matmul(out=u_ps[:], lhsT=mw[:], rhs=xg[:, c, :],
                                         start=(c == 0), stop=(c == ch - 1))
                        nc.tensor.matmul(out=s_ps[:], lhsT=mw[:], rhs=ones_t[:],
                                         start=(c == 0), stop=(c == ch - 1))

                    sden = pool.tile([128, 1], F32, tag="sden")
                    nc.vector.tensor_scalar(out=sden[:], in0=s_ps[:], scalar1=1e-30,
                                            scalar2=None, op0=ALU.max)
                    rs = pool.tile([128, 1], F32, tag="rs")
                    nc.vector.reciprocal(rs[:], sden[:])
                    a_sb = pool.tile([128, d], BF16, tag="asb")
                    nc.vector.tensor_scalar(out=a_sb[:], in0=u_ps[:], scalar1=rs[:],
                                            scalar2=None, op0=ALU.mult)
                    nc.sync.dma_start(cd["a_dram"][t * 128:(t + 1) * 128, :], a_sb[:])

            tc.strict_bb_all_engine_barrier()

            # ---- phase 3: projection + combine + relu --------------------
            with tc.tile_pool(name="proj", bufs=1) as ppool:
                at = {}
                for cv in ("l", "u"):
                    for h in range(2):
                        tl = ppool.tile([128, ownp], BF16, tag=f"at{cv}{h}")
                        nc.sync.dma_start(
                            tl[:], convs[cv]["a_dram"][:, h * 128:(h + 1) * 128],
                            transpose=True)
                        at[(cv, h)] = tl
                wsb = {}
                for nm, w_in in (("l", w_l), ("u", w_u), ("x", w_lin)):
                    tl = ppool.tile([128, 2, d], BF16, tag=f"w{nm}")
                    nc.sync.dma_start(tl[:, 0, :], w_in[0:128, :])
                    nc.sync.dma_start(tl[:, 1, :], w_in[128:256, :])
                    wsb[nm] = tl
                for t in range(ntiles):
                    xo = pool.tile([128, 2, 128], BF16, tag="xo")
                    nc.sync.dma_start(xo[:, 0, :], xot[0:128, t * 128:(t + 1) * 128])
                    nc.sync.dma_start(xo[:, 1, :], xot[128:256, t * 128:(t + 1) * 128])
                    o_ps = psum.tile([128, d], F32, tag="ops", bufs=2)
                    first = True
                    for cv in ("l", "u"):
                        for h in range(2):
                            nc.tensor.matmul(
                                out=o_ps[:],
                                lhsT=at[(cv, h)][:, t * 128:(t + 1) * 128],
                                rhs=wsb[cv][:, h, :],
                                start=first, stop=False)
                            first = False
                    nc.tensor.matmul(out=o_ps[:], lhsT=xo[:, 0, :],
                                     rhs=wsb["x"][:, 0, :], start=False, stop=False)
                    nc.tensor.matmul(out=o_ps[:], lhsT=xo[:, 1, :],
                                     rhs=wsb["x"][:, 1, :], start=False, stop=True)
                    o_sb = pool.tile([128, d], F32, tag="osb")
                    nc.scalar.activation(o_sb[:], o_ps[:], ACTF.Relu)
                    nc.sync.dma_start(out[t * 128:(t + 1) * 128, :], o_sb[:])

    import re as _re
    for blk in nc.m.functions[0].blocks:
        for inst in blk.instructions:
            if isinstance(inst, mybir.InstDMAGatherAnt):
                lane = None
                si = inst.sync_info
                ups = si.on_update if si is not None else []
                for u in ups:
                    m = _re.search(r"DMASW(\d+)", str(u.ant_name))
                    if m:
                        lane = int(m.group(1))
                        break
                if lane is not None:
                    inst.queue_num = lane % 4
    nc.finalize()
    return nc


# --------------------------------------------------------------------------
# top level
# --------------------------------------------------------------------------

def _prepare(x, lower_indices, lower_values, upper_indices, upper_values,
             W_lower, att_lower, W_upper, att_upper, W_lin,
             n_cores=8, split=32768):
    n, d = x.shape
    own = _ceil(n, n_cores)
    ntiles = _ceil(own, 128)
    ownp = ntiles * 128
    npad = _ceil(max(n_cores * ownp, n), 512) * 512

    meta_l, cores_l = _prep_conv(lower_indices, lower_values, n, n_cores, own,
                                 ntiles, split)
    meta_u, cores_u = _prep_conv(upper_indices, upper_values, n, n_cores, own,
                                 ntiles, split)

    xf = np.asarray(x, np.float32)
    x_pad = np.zeros((npad, d), np.float32)
    x_pad[:n] = xf
    x_tab = x_pad.astype(ml_dtypes.bfloat16)
    xt = np.ascontiguousarray(x_pad.T).astype(ml_dtypes.bfloat16)

    wl = np.asarray(W_lower, np.float32)
    wu = np.asarray(W_upper, np.float32)
    wlin = (np.asarray(W_lin, np.float32) * EPS)
    al = np.asarray(att_lower, np.float32)
    au = np.asarray(att_upper, np.float32)
    # wa4 cols -> sc_tab fields: [ss_l, ss_u, st_l, st_u]
    wa4 = np.stack([wl @ al[d:], wu @ au[d:], wl @ al[:d], wu @ au[:d]],
                   axis=1).astype(ml_dtypes.bfloat16)

    iota_np = np.broadcast_to(np.arange(128, dtype=np.float32), (128, 128)).copy()

    common = dict(x_tab=x_tab, xt=xt, wa4=wa4,
                  w_l=wl.astype(ml_dtypes.bfloat16),
                  w_u=wu.astype(ml_dtypes.bfloat16),
                  w_lin=wlin.astype(ml_dtypes.bfloat16),
                  iota_in=iota_np)
    in_maps = []
    for k in range(n_cores):
        m = dict(common)
        m["xot"] = np.ascontiguousarray(
            x_pad[k * own:k * own + ownp].T).astype(ml_dtypes.bfloat16)
        for cv, cores in (("l", cores_l), ("u", cores_u)):
            cdk = cores[k]
            m[f"idxa_{cv}"] = cdk["idxa"]
            m[f"idxb_{cv}"] = cdk["idxb"]
            m[f"toff_{cv}"] = cdk["toff"]
            m[f"val_{cv}"] = cdk["val"]
            m[f"toffi_{cv}"] = cdk["toffi"]
        in_maps.append(m)

    dims = dict(n=n, npad=npad, d=d, n_cores=n_cores, own=own, ntiles=ntiles,
                split=split)
    return dims, meta_l, meta_u, in_maps


def build_all(inputs, n_cores=8, split=32768):
    dims, meta_l, meta_u, in_maps = _prepare(**inputs, n_cores=n_cores,
                                             split=split)
    nc = _build_program(dims["n"], dims["npad"], dims["d"], dims["n_cores"],
                        dims["own"], dims["ntiles"], dims["split"],
                        meta_l, meta_u)
    return nc, in_maps, dims


def kernel(**inputs):
    nc, in_maps, dims = build_all(inputs)
    res = run_bass_kernel_spmd(nc, in_maps, list(range(dims["n_cores"])))
    outs = [res.results[k]["out"][:dims["own"]] for k in range(dims["n_cores"])]
    return np.concatenate(outs, axis=0)[:dims["n"]].astype(np.float32)
